# revision 20
# baseline (speedup 1.0000x reference)
"""Trainium2 Bass kernel for nn_NTM: 3-way embedding lookup -> GCP (relu
Hadamard-product projection) + factored Tucker branch -> scalar head.

out[n] = f1 . relu((ie*je*ke) @ w)
         + sum_ab (ie@U1E2)[ab] * (je@U2E1)[ab] * (ke@U3W2r)[ab]
         + fc_b
where U1E2[d,(a,b)]=U1[d,a], U2E1[d,(a,b)]=U2[d,b],
      U3W2r[d,(a,b)]=sum_c U3[d,c]*W2[a,b,c], W2=fc_w[0,100:].reshape(5,5,5)

Sharding: data-parallel over n across 8 cores (16384 samples each). Each
core's embedding tables are compacted host-side (np.unique) so local gather
indices fit int16 (dma_gather requirement); the device does the full
per-sample gather from DRAM.

Per-core dataflow, per 512-sample chunk:
  dma_gather u/i/time rows -> [128, slots, 64] sample-major SBUF -> PE
  pair-transposes (two consecutive slots of one table -> parity-interleaved
  d-major) -> ACT/DVE copy to SBUF -> DVE Hadamard product computed in
  d-major (u/i/t sections share partitions at different free offsets) ->
  per-parity K=128 projection matmuls with zero-masked weight halves
  (col-group packed: four chunks share each Tucker-factor PSUM bank) ->
  DVE product of the three Tucker factors -> two K-split reduction matmuls
  accumulate 8 chunks into one [8, 512] PSUM tile -> bias + store with a
  parity-unpermuting DMA.
"""

import numpy as np
from contextlib import ExitStack

N_FULL = 131072
N_CORES = 8
NPC = N_FULL // N_CORES  # 16384
D = 64
GSLAB = 1024  # gather / Hadamard-product granularity
CHUNK = 512   # compute granularity (PSUM bank free-dim limit, f32)
TT_ROWS = 512  # padded time-table rows


def _f32(x):
    return np.ascontiguousarray(np.asarray(x, dtype=np.float32))


_PATCHED = False
TRACE = False          # set by test harness to capture an NTFF profile
LAST_RESULTS = None    # BassKernelResults of the most recent kernel() call


def _patch_tile_drain():
    """Workaround: this walrus build accepts at most 1 sem-wait condition per
    instruction, but Tile's kernel-tail drain waits on one sem per logical
    proc used. The drain and preceding NOPs run on the same engine (SP) in
    program order, so spreading the waits across leading NOPs is equivalent."""
    global _PATCHED
    if _PATCHED:
        return
    _PATCHED = True
    import concourse.tile as tile
    import concourse.mybir as mybir
    from concourse.vector_clock import ScopedClock

    def _drain_and_barrier(self, tick_clock, wait_clock):
        nc = self.nc
        probe = nc.sync.nop(nofuse=True, hint="drain_wait_split")
        wait_clock.add_sem_waits(
            probe.ins, ScopedClock({None: tick_clock.global_clock}))
        waits = list(probe.ins.sync_info.on_wait) if probe.ins.sync_info else []
        chunks = [waits[i:i + 1] for i in range(len(waits))] or [[]]
        probe.ins.sync_info = mybir.SyncInfo(on_wait=chunks[0], on_update=[])
        for chunk in chunks[1:]:
            nop = nc.sync.nop(nofuse=True, hint="drain_wait_split")
            nop.ins.sync_info = mybir.SyncInfo(on_wait=chunk, on_update=[])
        nc.sync.drain()
        nc.all_engine_barrier()
        assert self.sems is not None
        popped = nc._tile_sem_poison_stack.pop()
        assert popped is self._sem_poison
        nc.clear_and_free_semaphores(list(self.sems.allocated().values()))
        nc.all_engine_barrier()

    tile.TileContext._drain_and_barrier = _drain_and_barrier


def build_program(nc, npc=NPC):
    import concourse.tile as tile
    from concourse import mybir

    f32 = mybir.dt.float32
    i16 = mybir.dt.int16
    AF = mybir.ActivationFunctionType

    nchunks = npc // CHUNK
    assert npc % 4096 == 0  # 8-chunk fin groups

    ut = nc.dram_tensor("ut", [npc, D], f32, kind="ExternalInput")
    itab = nc.dram_tensor("itab", [npc, D], f32, kind="ExternalInput")
    tt = nc.dram_tensor("tt", [TT_ROWS, D], f32, kind="ExternalInput")
    idxu = nc.dram_tensor("idxu", [128, npc // 16], i16, kind="ExternalInput")
    idxi = nc.dram_tensor("idxi", [128, npc // 16], i16, kind="ExternalInput")
    idxt = nc.dram_tensor("idxt", [128, npc // 16], i16, kind="ExternalInput")
    wA = nc.dram_tensor("wA", [128, 64], f32, kind="ExternalInput")
    wB = nc.dram_tensor("wB", [128, 64], f32, kind="ExternalInput")
    wH = nc.dram_tensor("wH", [128, 64], f32, kind="ExternalInput")
    wG = nc.dram_tensor("wG", [128, 256], f32, kind="ExternalInput")
    wFa = nc.dram_tensor("wFa", [128, 64], f32, kind="ExternalInput")
    wFb = nc.dram_tensor("wFb", [100, 64], f32, kind="ExternalInput")
    ident = nc.dram_tensor("ident", [128, 128], f32, kind="ExternalInput")
    biasb = nc.dram_tensor("biasb", [8, 1], f32, kind="ExternalInput")
    y = nc.dram_tensor("y", [nchunks, CHUNK], f32, kind="ExternalOutput")

    with tile.TileContext(nc) as tc, ExitStack() as ctx:
        const = ctx.enter_context(tc.tile_pool(name="const", bufs=1))

        def load_const(dram, shape, dtype=f32):
            t = const.tile(shape, dtype, tag=dram.name)
            nc.sync.dma_start(t[:], dram[:])
            return t

        idxu_sb = load_const(idxu, [128, npc // 16], i16)
        idxi_sb = load_const(idxi, [128, npc // 16], i16)
        idxt_sb = load_const(idxt, [128, npc // 16], i16)
        wA_sb = load_const(wA, [128, 64])
        wB_sb = load_const(wB, [128, 64])
        wH_sb = load_const(wH, [128, 64])
        wG_sb = load_const(wG, [128, 256])
        wFa_sb = load_const(wFa, [128, 64])
        wFb_sb = load_const(wFb, [100, 64])
        id_sb = load_const(ident, [128, 128])
        bias_sb = load_const(biasb, [8, 1])

        gpool = ctx.enter_context(tc.tile_pool(name="gath", bufs=4))
        t12pool = ctx.enter_context(tc.tile_pool(name="t12", bufs=2))
        t34pool = ctx.enter_context(tc.tile_pool(name="t34", bufs=2))
        relupool = ctx.enter_context(tc.tile_pool(name="relu", bufs=2))
        sbBpool = ctx.enter_context(tc.tile_pool(name="sbB", bufs=2))
        t1pool = ctx.enter_context(tc.tile_pool(name="t1", bufs=2))
        tuckpool = ctx.enter_context(tc.tile_pool(name="tuck", bufs=2))
        outpool = ctx.enter_context(tc.tile_pool(name="outp", bufs=2))

        ps12 = ctx.enter_context(tc.tile_pool(name="ps12", bufs=2, space="PSUM"))
        ps34 = ctx.enter_context(tc.tile_pool(name="ps34", bufs=1, space="PSUM"))
        psg = ctx.enter_context(tc.tile_pool(name="psg", bufs=1, space="PSUM"))
        psA = ctx.enter_context(tc.tile_pool(name="psA", bufs=1, space="PSUM"))
        psB = ctx.enter_context(tc.tile_pool(name="psB", bufs=1, space="PSUM"))
        psH = ctx.enter_context(tc.tile_pool(name="psH", bufs=1, space="PSUM"))
        psF = ctx.enter_context(tc.tile_pool(name="psF", bufs=1, space="PSUM"))

        G_ui = G_tp = None
        for fgrp in range(nchunks // 8):
            fin_ps = psF.tile([8, CHUNK], f32)
            for m in range(2):
                A_ps = psA.tile([128, CHUNK], f32)
                B_ps = psB.tile([128, CHUNK], f32)
                H_ps = psH.tile([128, CHUNK], f32)
                for jj in range(4):
                    c = fgrp * 8 + m * 4 + jj
                    v = c % 8
                    if c % 2 == 0:
                        g = c // 2
                        G_ui = gpool.tile([128, 16, D], f32, tag="gui")
                        G_tp = gpool.tile([128, 8, D], f32, tag="gtp")
                        ii = idxu_sb[:, 64 * g:64 * (g + 1)]
                        nc.gpsimd.dma_gather(
                            G_ui[:, 0:8, :], ut[:, :], ii, GSLAB, GSLAB, D,
                            elem_step=D)
                        ii = idxi_sb[:, 64 * g:64 * (g + 1)]
                        nc.gpsimd.dma_gather(
                            G_ui[:, 8:16, :], itab[:, :], ii, GSLAB, GSLAB, D,
                            elem_step=D)
                        ii = idxt_sb[:, 64 * g:64 * (g + 1)]
                        nc.gpsimd.dma_gather(
                            G_tp[:, 0:8, :], tt[:, :], ii, GSLAB, GSLAB, D,
                            elem_step=D)
                    half = c % 2
                    # Pair-transposes of two consecutive slots of one table:
                    # in [128 samp, 128] -> out [(parity e, d), 128 lanes].
                    # Chunk-local sample = 128*(2p+e) + lane  (p = pair idx).
                    # T_ui psum free: u-table at 0:256, i-table at 256:512;
                    # T_tp: time at 0:256, Hadamard product at 256:512.
                    T12_ps = ps12.tile([128, CHUNK], f32)
                    T34_ps = ps34.tile([128, CHUNK // 2], f32)
                    ui_f = G_ui[:, :, :].rearrange("p t d -> p (t d)")
                    tp_f = G_tp[:, :, :].rearrange("p t d -> p (t d)")
                    for p in range(2):
                        s = 4 * half + 2 * p
                        nc.tensor.transpose(
                            T12_ps[:, 128 * p:128 * (p + 1)],
                            ui_f[:, 64 * s:64 * s + 128], id_sb[:])
                        nc.tensor.transpose(
                            T12_ps[:, 256 + 128 * p:256 + 128 * (p + 1)],
                            ui_f[:, 64 * (8 + s):64 * (8 + s) + 128], id_sb[:])
                        nc.tensor.transpose(
                            T34_ps[:, 128 * p:128 * (p + 1)],
                            tp_f[:, 64 * s:64 * s + 128], id_sb[:])
                    T12_sb = t12pool.tile([128, CHUNK], f32)
                    nc.scalar.activation(T12_sb[:], T12_ps[:], AF.Copy)
                    T34_sb = t34pool.tile([128, CHUNK // 2], f32)
                    nc.vector.tensor_copy(T34_sb[:], T34_ps[:])
                    # Hadamard product in d-major: u and i sections share
                    # partitions at free offsets 0:256 / 256:512
                    pt_sb = ptpool.tile([128, CHUNK // 2], f32)
                    nc.vector.tensor_mul(
                        pt_sb[:], T12_sb[:, 0:256], T12_sb[:, 256:512])
                    nc.vector.tensor_mul(pt_sb[:], pt_sb[:], T34_sb[:])
                    gcp_ps = psg.tile([128, CHUNK], f32)
                    relu_sb = relupool.tile([100, CHUNK], f32)
                    for e in range(2):
                        # parity selected by zero-masked weight halves; all
                        # matmuls K=128 from partition 0 (row-tiled matmuls
                        # crash this runtime)
                        fsl = slice(256 * e, 256 * (e + 1))
                        nc.tensor.matmul(
                            A_ps[32 * jj:32 * (jj + 1), fsl],
                            wA_sb[:, 32 * e:32 * (e + 1)], T12_sb[:, 0:256],
                            start=True, stop=True, tile_position=(0, 32 * jj))
                        nc.tensor.matmul(
                            B_ps[32 * jj:32 * (jj + 1), fsl],
                            wB_sb[:, 32 * e:32 * (e + 1)], T12_sb[:, 256:512],
                            start=True, stop=True, tile_position=(0, 32 * jj))
                        nc.tensor.matmul(
                            H_ps[32 * jj:32 * (jj + 1), fsl],
                            wH_sb[:, 32 * e:32 * (e + 1)], T34_sb[:, :],
                            start=True, stop=True, tile_position=(0, 32 * jj))
                        nc.tensor.matmul(
                            gcp_ps[:, fsl],
                            wG_sb[:, 128 * e:128 * (e + 1)], pt_sb[:, :],
                            start=True, stop=True, tile_position=(0, 0))
                    nc.scalar.activation(relu_sb[:], gcp_ps[0:100, :], AF.Relu)
                    nc.tensor.matmul(
                        fin_ps[:, :], wFb_sb[:, 8 * v:8 * (v + 1)], relu_sb[:],
                        start=(v == 0), stop=False)
                # Tucker factor product for the macro's 4 chunks
                sbB = sbBpool.tile([128, CHUNK], f32)
                nc.scalar.activation(sbB[:], B_ps[:], AF.Copy)
                t1 = t1pool.tile([128, CHUNK], f32)
                nc.vector.tensor_mul(t1[:], A_ps[:], sbB[:])
                tuck = tuckpool.tile([128, CHUNK], f32)
                nc.vector.tensor_mul(tuck[:], t1[:], H_ps[:])
                for jj in range(4):
                    c = fgrp * 8 + m * 4 + jj
                    v = c % 8
                    nc.tensor.matmul(
                        fin_ps[:, :], wFa_sb[:, 8 * v:8 * (v + 1)], tuck[:, :],
                        start=False, stop=(v == 7), tile_position=(0, 0))
            out_sb = outpool.tile([8, CHUNK], f32)
            nc.scalar.activation(out_sb[:], fin_ps[:], AF.Identity, bias=bias_sb[:])
            # lane f = 256e + 128p + l holds sample 128*(2p+e) + l; the
            # rearranged DRAM view iterates (e, p, l) to undo the permute.
            y_v = y[fgrp * 8:(fgrp + 1) * 8, :].rearrange(
                "r (p e l) -> r e p l", p=2, e=2)
            nc.sync.dma_start(y_v[:, 0, :, :], out_sb[:, 0:256])
            nc.sync.dma_start(y_v[:, 1, :, :], out_sb[:, 256:512])
    return nc


def make_weights(w, U1, U2, U3, fc_w, fc_b):
    """Host-folded weight matrices (shared by all cores)."""
    w = _f32(w); U1 = _f32(U1); U2 = _f32(U2); U3 = _f32(U3)
    fc_w = _f32(fc_w).reshape(-1)
    f1 = fc_w[:100]
    W2 = fc_w[100:].reshape(5, 5, 5)
    wA1 = np.zeros((64, 32), np.float32)
    wB1 = np.zeros((64, 32), np.float32)
    wH1 = np.zeros((64, 32), np.float32)
    for a in range(5):
        for b in range(5):
            wA1[:, a * 5 + b] = U1[:, a]
            wB1[:, a * 5 + b] = U2[:, b]
            wH1[:, a * 5 + b] = U3 @ W2[a, b, :]
    # All matmuls run K=128 from partition 0; the sample-parity half is
    # selected by zero-masking the unused 64 weight rows (row-tiled matmuls
    # crash this runtime). Layout: variant e at cols [e*M : (e+1)*M].
    z = np.zeros_like(wA1)
    wA = np.vstack([np.hstack([wA1, z]), np.hstack([z, wA1])])  # [128, 64]
    wB = np.vstack([np.hstack([wB1, z]), np.hstack([z, wB1])])
    wH = np.vstack([np.hstack([wH1, z]), np.hstack([z, wH1])])
    wG1 = np.zeros((64, 128), np.float32)
    wG1[:, 0:100] = w
    zg = np.zeros_like(wG1)
    wG = np.vstack([np.hstack([wG1, zg]), np.hstack([zg, wG1])])  # [128, 256]
    # fin_a: variant v reduces tuck rows 32*(v%4)..+25 into output row v
    wFa = np.zeros((128, 64), np.float32)
    wFb = np.zeros((100, 64), np.float32)
    for v in range(8):
        wFa[32 * (v % 4):32 * (v % 4) + 25, 8 * v + v] = 1.0
        wFb[:, 8 * v + v] = f1
    biasb = np.full((8, 1), np.float32(np.asarray(fc_b).reshape(-1)[0]),
                    np.float32)
    ident = np.eye(128, dtype=np.float32)
    return dict(wA=wA, wB=wB, wH=wH, wG=wG, wFa=wFa, wFb=wFb, biasb=biasb,
                ident=ident)


def _wrap_idx(local_idx):
    """dma_gather index layout: [128, n/16] int16; sample n at
    (partition n%16, col n//16), replicated across the 8 partition groups."""
    n = local_idx.shape[0]
    wrapped = local_idx.astype(np.int16).reshape(n // 16, 16).T  # [16, n/16]
    return np.ascontiguousarray(np.tile(wrapped, (8, 1)))


def make_core_inputs(i_sh, j_sh, k_sh, user_emb, item_emb, time_emb, shared):
    npc = i_sh.shape[0]
    uq_u, inv_u = np.unique(i_sh, return_inverse=True)
    uq_i, inv_i = np.unique(j_sh, return_inverse=True)
    ut = np.zeros((npc, D), np.float32)
    ut[: len(uq_u)] = user_emb[uq_u]
    itab = np.zeros((npc, D), np.float32)
    itab[: len(uq_i)] = item_emb[uq_i]
    tt = np.zeros((TT_ROWS, D), np.float32)
    tt[: time_emb.shape[0]] = time_emb
    m = dict(shared)
    m.update(
        ut=ut, itab=itab, tt=tt,
        idxu=_wrap_idx(inv_u), idxi=_wrap_idx(inv_i),
        idxt=_wrap_idx(k_sh.astype(np.int64)),
    )
    return m


def kernel(i_input, j_input, k_input, user_emb, item_emb, time_emb,
           w, U1, U2, U3, fc_w, fc_b):
    from concourse import bacc
    from concourse.bass_utils import run_bass_kernel_spmd
    _patch_tile_drain()

    i_input = np.asarray(i_input).astype(np.int64)
    j_input = np.asarray(j_input).astype(np.int64)
    k_input = np.asarray(k_input).astype(np.int64)
    user_emb = _f32(user_emb)
    item_emb = _f32(item_emb)
    time_emb = _f32(time_emb)

    n = i_input.shape[0]
    assert n == N_FULL
    npc = n // N_CORES

    shared = make_weights(w, U1, U2, U3, fc_w, fc_b)
    in_maps = []
    for c in range(N_CORES):
        sl = slice(c * npc, (c + 1) * npc)
        in_maps.append(make_core_inputs(
            i_input[sl], j_input[sl], k_input[sl],
            user_emb, item_emb, time_emb, shared))

    nc = bacc.Bacc("TRN2", target_bir_lowering=False, debug=False,
                   num_devices=N_CORES)
    build_program(nc, npc)
    nc.compile()

    res = None
    for attempt in range(3):
        try:
            res = run_bass_kernel_spmd(
                nc, in_maps, core_ids=list(range(N_CORES)), trace=TRACE)
            break
        except Exception:
            # a prior workload (e.g. the jax reference running on these
            # devices) can leave a core wedged; retry after recovery
            if attempt == 2:
                raise
    global LAST_RESULTS
    LAST_RESULTS = res
    out = np.concatenate(
        [res.results[c]["y"].reshape(-1) for c in range(N_CORES)])
    return out.astype(np.float32)
